# revision 10
# baseline (speedup 1.0000x reference)
"""ECE loss kernel for Trainium2 (Bass/Tile), data-parallel over 8 NeuronCores.

Math (per sample row of logits[N, C]):
  conf = max softmax(x) = max(E) / sum(E),  E = exp(x)
  acc  = (argmax(x) == label)  via  exp(g) == max(E), g = x[i, label_i]
  ece  = sum_b |conf_sum[b] - acc_sum[b]| / N   over 15 real bins

Per-core device work (125k rows as [125 partitions x 1000 samples x 100 cls]),
balanced across ALL engines (the previous version put everything on DVE):
  - DMA   (sync HWDGE only): 13 tiles, up to 5 MB each
  - ACT   : E = exp(x) in place; later all per-bin statistics via
            activation(Relu/Sign, bias=-C, accum_out=...) which gives a free
            per-partition sum of the activated values
  - DVE   : rowmax(E); rowsum for the small lead tiles; final rowsum over 25
            for the big tiles; recip/eq/mul/stt epilogue per chunk
  - GpSimd: two pairwise-ADD tree levels (100->50->25) in place on each big
            tile, via tensor_tensor(add) -- runs after DVE's rowmax read
            (Pool TT supports add/mult but not max)

Per-bin statistics (accumulated per chunk of samples so they overlap the
main loop instead of forming a serial tail):
  wt'(Cb) = sum relu(conf - Cb)        (15 ops: Cb in {0} + C_0..C_13)
  q(Cb)   = sum sign(conf - Cb)        (14 ops: #gt - #lt, ties ~0)
  q'(Tb)  = sum sign(v - Tb), v = 2*acc - conf, Tb = 2-C_b  (15 ops, last is
            Tb=0.5 which counts acc exactly: v in [-1,0) u [1,2))
Host recovers:
  T = wt'(0);  #le_b = (N - q_b)/2;  S_b = T - wt'_b - C_b*(N - #le_b)
  A_b = (N + q'_b)/2   (cumulative acc counts);  diffs give per-bin sums.
C_b is the exact f32 boundary: the largest f32 y with f32(15*y) <= b+1, so
binning matches the reference's ceil(conf*15) up to ~1-sample tie effects
(~1e-6 relative on the final ECE).
"""

import os

import numpy as np

import concourse.bass as bass
import concourse.mybir as mybir
import concourse.tile as tile
from concourse.bass_utils import run_bass_kernel_spmd

F32 = mybir.dt.float32
ALU = mybir.AluOpType
AX = mybir.AxisListType
ACTF = mybir.ActivationFunctionType

N = 1_000_000
C = 100
NCORES = 8
ROWS = N // NCORES          # 125000 rows per core
P = 125                     # SBUF partitions used
SPP = ROWS // P             # 1000 samples per partition

SIZES = [12, 13, 25, 50] + [100] * 9          # samples/partition per tile
CHUNKS = [(0, 600), (600, 900), (900, 1000)]  # binning chunks
CHUNK_LAST_TILE = [8, 11, 12]                 # last tile index of each chunk
NSLOT = 44                                    # 15 wt + 14 nn + 15 av

LAST_RESULTS = None         # stashed BassKernelResults for test harness


def _bin_thresholds():
    """C_b = largest f32 y such that f32(15*y) <= b+1, for b = 0..14."""
    thr = []
    for b in range(15):
        tgt = np.float32(b + 1)

        def f(v):
            return np.float32(np.float32(15.0) * v)

        y = np.float32((b + 1) / 15.0)
        if f(y) <= tgt:
            while True:
                y2 = np.nextafter(y, np.float32(np.inf))
                if f(y2) <= tgt:
                    y = y2
                else:
                    break
        else:
            while f(y) > tgt:
                y = np.nextafter(y, np.float32(-np.inf))
        thr.append(np.float32(y))
    return thr


THR = _bin_thresholds()                       # 15 values, b = 0..14

# bias constants shipped as a tiny input tensor (the const-AP pool only has
# 0.0/1.0 pre-registered):  [0] = 0.0 (wt base),  [1+b] = -C_b (wt/nn),
# [15+b] = C_b - 2 = -(2-C_b) (av),  [29] = -0.5 (acc count)
NCONST = 30
CVEC = np.zeros(NCONST, np.float32)
for _b in range(14):
    CVEC[1 + _b] = -THR[_b]
    CVEC[15 + _b] = np.float32(THR[_b] - np.float32(2.0))
CVEC[29] = np.float32(-0.5)


def _fix_sync(nc):
    """Instruction encodings only carry 2 sync-command slots (completion
    update takes one), so every instruction should hold <= 1 wait.  Tile's
    sem emission is not transitively minimal, so: (1) drop waits implied
    transitively through other waits / same-engine program order; (2) split
    any leftover multi-wait instruction into a chain of presync drains."""
    import bisect
    import re

    import bass_rust as _br

    TICK = re.compile(r"^(Activation|DVE|PE|Pool|SP|DMAHW\d+|DMASW\d+)_\d+$")
    ASYNC_T = {"InstDMACopy", "InstTriggerDma"}

    insts = []
    for bb in nc.m.functions[0].blocks:
        for ins in bb.instructions:
            insts.append(ins)
    n = len(insts)

    # producer map: tick sem -> sorted cumulative values + producing inst idx
    prod_vals, prod_idx = {}, {}
    own_updates = [[] for _ in range(n)]
    cum = {}
    for idx, ins in enumerate(insts):
        si = ins.sync_info
        if si is None:
            continue
        for u in si.on_update:
            nm = u.ant_name
            if not nm or not TICK.match(nm) or u.update_mode != "sem-inc":
                continue
            v = cum.get(nm, 0) + (u.update_value or 1)
            cum[nm] = v
            prod_vals.setdefault(nm, []).append(v)
            prod_idx.setdefault(nm, []).append(idx)
            own_updates[idx].append((nm, v))

    def producer(nm, val):
        vs = prod_vals.get(nm)
        if not vs:
            return None
        k = bisect.bisect_left(vs, val)
        if k >= len(vs):
            return None
        return prod_idx[nm][k]

    prev_idx = [None] * n
    last = {}
    for idx, ins in enumerate(insts):
        e = str(getattr(ins, "engine", None))
        prev_idx[idx] = last.get(e)
        last[e] = idx

    # before[i]: sem clock guaranteed when inst i issues (incl its waits)
    # after[i]: clock guaranteed when inst i COMPLETES (incl own updates)
    before = [None] * n
    after = [None] * n

    def wait_producers(i):
        si = insts[i].sync_info
        out = []
        for w in (si.on_wait if si else []):
            pi = None
            if w.ant_name and TICK.match(w.ant_name):
                pi = producer(w.ant_name, w.wait_value)
                if pi == i:
                    pi = None
            out.append((w, pi))
        return out

    def compute(idx):
        stack = [idx]
        while stack:
            i = stack[-1]
            if after[i] is not None:
                stack.pop()
                continue
            deps = []
            p = prev_idx[i]
            if p is not None and after[p] is None:
                deps.append(p)
            wps = wait_producers(i)
            for w, pi in wps:
                if pi is not None and after[pi] is None:
                    deps.append(pi)
            if deps:
                stack.extend(deps)
                continue
            stack.pop()
            c = {}
            if p is not None:
                src = before[p] if type(insts[p]).__name__ in ASYNC_T else after[p]
                for s, v in src.items():
                    if c.get(s, -1) < v:
                        c[s] = v
            for w, pi in wps:
                if pi is not None:
                    for s, v in after[pi].items():
                        if c.get(s, -1) < v:
                            c[s] = v
                if w.ant_name and TICK.match(w.ant_name):
                    if c.get(w.ant_name, -1) < w.wait_value:
                        c[w.ant_name] = w.wait_value
            before[i] = c
            a = dict(c)
            for nm, v in own_updates[i]:
                if a.get(nm, -1) < v:
                    a[nm] = v
            after[i] = a

    for i in range(n):
        compute(i)

    # pass 1: transitive reduction of each instruction's wait list
    for i, ins in enumerate(insts):
        si = ins.sync_info
        if si is None or len(si.on_wait) <= 1:
            continue
        if type(ins).__name__ == "InstEventSemaphore":
            continue
        waits = list(si.on_wait)
        p = prev_idx[i]
        base = {}
        if p is not None:
            src = before[p] if type(insts[p]).__name__ in ASYNC_T else after[p]
            base.update(src)
        closures = []
        for w in waits:
            cl = {}
            if w.ant_name and TICK.match(w.ant_name):
                pi = producer(w.ant_name, w.wait_value)
                if pi is not None and pi != i:
                    cl.update(after[pi])
                if cl.get(w.ant_name, -1) < w.wait_value:
                    cl[w.ant_name] = w.wait_value
            closures.append(cl)
        kept = []
        kept_cl = dict(base)
        for j, w in enumerate(waits):
            nm = w.ant_name
            if not (nm and TICK.match(nm)):
                kept.append(w)
                continue
            cov = dict(kept_cl)
            for j2 in range(j + 1, len(waits)):
                for s, v in closures[j2].items():
                    if cov.get(s, -1) < v:
                        cov[s] = v
            if cov.get(nm, -1) >= w.wait_value:
                continue
            kept.append(w)
            for s, v in closures[j].items():
                if kept_cl.get(s, -1) < v:
                    kept_cl[s] = v
        if len(kept) != len(waits):
            si.on_wait = kept
            ins.sync_info = si

    # pass 2: split any instruction still carrying > 1 wait into a chain of
    # same-engine presync drains (each drain fits a single sync command)
    for bb in nc.m.functions[0].blocks:
        while True:
            insns = list(bb.instructions)
            target = None
            for idx, ins in enumerate(insns):
                si = ins.sync_info
                if si is None:
                    continue
                if len(si.on_wait) > 1:
                    target = (idx, ins)
                    break
            if target is None:
                break
            idx, ins = target
            si = ins.sync_info
            waits = list(si.on_wait)
            if type(ins).__name__ == "InstDrain":
                room = max(0, 1 - len(si.on_update))
            else:
                room = 1
            keep, extra = waits[len(waits) - room:], waits[: len(waits) - room]
            pos = idx
            for i, w in enumerate(extra):
                nd = mybir.InstDrain(
                    name=f"{ins.name}-presync{i}", ins=[], outs=[],
                    bass_is_fusable=False,
                )
                nd.engine = ins.engine
                nd.sync_info = _br.SyncInfo(on_wait=[w], on_update=[])
                nc.register_instruction(nd, overwrite=True)
                bb.instructions.insert(pos, nd)
                pos += 1
            si.on_wait = keep
            ins.sync_info = si


def _build():
    nc = bass.Bass(trn_type="TRN2")
    x = nc.dram_tensor("x", [P, SPP * C], F32, kind="ExternalInput")
    g = nc.dram_tensor("g", [P, SPP], F32, kind="ExternalInput")
    cst = nc.dram_tensor("cst", [P, NCONST], F32, kind="ExternalInput")
    st = nc.dram_tensor("st", [P, 3 * NSLOT], F32, kind="ExternalOutput")

    X = x[:, :].rearrange("p (k c) -> p k c", c=C)  # [125, 1000, 100]

    with tile.TileContext(nc) as tc:
        with (
            tc.tile_pool(name="xin", bufs=4) as xin,
            tc.tile_pool(name="persist", bufs=1) as persist,
        ):
            # per-chunk persistent buffers (separate tiles so later-tile
            # writes never alias earlier chunks' binning reads)
            m_ch = [persist.tile([P, hi - lo], F32, tag=f"m{i}", name=f"m{i}")
                    for i, (lo, hi) in enumerate(CHUNKS)]
            s_ch = [persist.tile([P, hi - lo], F32, tag=f"s{i}", name=f"s{i}")
                    for i, (lo, hi) in enumerate(CHUNKS)]
            eg = persist.tile([P, SPP], F32)
            dump = persist.tile([P, 600], F32)
            stats = persist.tile([P, 3, NSLOT], F32)
            cst_sb = persist.tile([P, NCONST], F32)

            nc.scalar.dma_start(out=eg[:, :], in_=g[:, :])
            nc.scalar.dma_start(out=cst_sb[:, :], in_=cst[:, :])

            off = 0
            ci = 0
            for t, k in enumerate(SIZES):
                lo, hi = CHUNKS[ci]
                ll = off - lo          # local offset within chunk buffers
                sl = slice(ll, ll + k)
                off += k
                m_c, s_c = m_ch[ci], s_ch[ci]

                xt = xin.tile([P, 100, C], F32, tag="xt")
                nc.sync.dma_start(out=xt[:, :k, :], in_=X[:, off - k:off, :])
                nc.scalar.activation(xt[:, :k, :], xt[:, :k, :], ACTF.Exp)
                if t == 2:
                    nc.scalar.activation(eg[:, :], eg[:, :], ACTF.Exp)
                nc.vector.reduce_max(out=m_c[:, sl], in_=xt[:, :k, :], axis=AX.X)
                if t < 4:
                    nc.vector.reduce_sum(
                        out=s_c[:, sl], in_=xt[:, :k, :], axis=AX.X
                    )
                else:
                    # pairwise ADD tree on GpSimd (Pool TT supports add, not
                    # max), in place after the DVE rowmax has read the tile,
                    # then a 25-wide DVE reduce finishes the row sum
                    nc.gpsimd.tensor_tensor(
                        xt[:, :k, 0:50], xt[:, :k, 0:50], xt[:, :k, 50:100],
                        op=ALU.add,
                    )
                    nc.gpsimd.tensor_tensor(
                        xt[:, :k, 0:25], xt[:, :k, 0:25], xt[:, :k, 25:50],
                        op=ALU.add,
                    )
                    nc.vector.reduce_sum(
                        out=s_c[:, sl], in_=xt[:, :k, 0:25], axis=AX.X
                    )

                if t == CHUNK_LAST_TILE[ci]:
                    # chunk epilogue on DVE: r = 1/S; acc = (exp(g) == maxE);
                    # conf = maxE * r; v = 2*acc - conf
                    L = hi - lo
                    egc = eg[:, lo:hi]
                    nc.vector.reciprocal(s_c[:, :L], s_c[:, :L])
                    nc.vector.tensor_tensor(
                        egc, egc, m_c[:, :L], op=ALU.is_equal
                    )
                    nc.vector.tensor_mul(m_c[:, :L], m_c[:, :L], s_c[:, :L])
                    nc.vector.scalar_tensor_tensor(
                        s_c[:, :L], egc, 2.0, m_c[:, :L],
                        op0=ALU.mult, op1=ALU.subtract,
                    )
                    ci = min(ci + 1, len(CHUNKS) - 1)

            # binning: all on ACT via accum_out; emitted after the main loop
            # so the scheduler treats them as gap fillers for ACT
            for cidx, (lo, hi) in enumerate(CHUNKS):
                L = hi - lo
                conf = m_ch[cidx][:, :L]
                v = s_ch[cidx][:, :L]
                dmp = dump[:, 0:L]
                # wt: relu(conf - C_b) for C in {0} + C_0..C_13
                for j in range(15):
                    bias = cst_sb[:, 0:1] if j == 0 else cst_sb[:, j:j + 1]
                    nc.scalar.activation(
                        dmp, conf, ACTF.Relu, bias=bias,
                        accum_out=stats[:, cidx:cidx + 1, j:j + 1],
                    )
                # nn: sign(conf - C_b), b = 0..13
                for j in range(14):
                    nc.scalar.activation(
                        dmp, conf, ACTF.Sign, bias=cst_sb[:, 1 + j:2 + j],
                        accum_out=stats[:, cidx:cidx + 1, 15 + j:16 + j],
                    )
                # av: sign(v - (2-C_b)) b = 0..13, then sign(v - 0.5)
                for j in range(15):
                    bias = cst_sb[:, 15 + j:16 + j] if j < 14 else cst_sb[:, 29:30]
                    nc.scalar.activation(
                        dmp, v, ACTF.Sign, bias=bias,
                        accum_out=stats[:, cidx:cidx + 1, 29 + j:30 + j],
                    )

            nc.sync.dma_start(out=st[:, :], in_=stats[:, :, :])

    _fix_sync(nc)
    return nc


_NC_CACHE = {}


def _get_nc():
    if "nc" not in _NC_CACHE:
        _NC_CACHE["nc"] = _build()
    return _NC_CACHE["nc"]


def kernel(logits, labels):
    global LAST_RESULTS
    logits = np.ascontiguousarray(np.asarray(logits), dtype=np.float32)
    labels_i = np.asarray(labels).astype(np.int64)
    assert logits.shape == (N, C), logits.shape

    # host-side gather of the label logit (1% of input bytes; the heavy
    # softmax/max/binning all happen on device)
    gvals = logits[np.arange(N), labels_i].astype(np.float32)

    in_maps = []
    for c in range(NCORES):
        sl = slice(c * ROWS, (c + 1) * ROWS)
        in_maps.append(
            {
                "x": logits[sl].reshape(P, SPP * C),
                "g": gvals[sl].reshape(P, SPP),
                "cst": np.tile(CVEC, (P, 1)),
            }
        )

    trace = bool(int(os.environ.get("ECE_TRACE", "0")))
    res = run_bass_kernel_spmd(
        _get_nc(), in_maps, core_ids=list(range(NCORES)), trace=trace
    )
    LAST_RESULTS = res

    tot = np.zeros((3, NSLOT), np.float64)
    for out in res.results:
        tot += out["st"].astype(np.float64).reshape(P, 3, NSLOT).sum(axis=0)
    slots = tot.sum(axis=0)                    # [44] summed over chunks

    wtv = slots[0:15]                          # relu sums: C=0, C_0..C_13
    q_nn = slots[15:29]                        # sign sums vs C_0..C_13
    q_av = slots[29:44]                        # sign sums on v; [14]=acc cnt

    thr64 = np.array([np.float64(t) for t in THR])
    T = wtv[0]
    nle = (N - q_nn) / 2.0                     # #(conf <= C_b), b=0..13
    ngt = N - nle
    S = np.empty(16)
    S[0:14] = T - wtv[1:15] - thr64[0:14] * ngt
    S[14] = T
    S[15] = T
    A = np.empty(16)
    A[0:14] = (N + q_av[0:14]) / 2.0
    A[14] = (N + q_av[14]) / 2.0               # total #acc (v >= 0.5)
    A[15] = A[14]
    conf_sum = np.diff(S, prepend=0.0)
    acc_sum = np.diff(A, prepend=0.0)
    ece = np.abs(conf_sum - acc_sum).sum() / N
    return np.array([ece], dtype=np.float32)


# revision 16
# speedup vs baseline: 1.0281x; 1.0281x over previous
"""ECE loss kernel for Trainium2 (Bass/Tile), data-parallel over 8 NeuronCores.

Math (per sample row of logits[N, C]):
  conf = max softmax(x) = max(E) / sum(E),  E = exp(x)
  acc  = (argmax(x) == label)  via  exp(g) == max(E), g = x[i, label_i]
  ece  = sum_b |conf_sum[b] - acc_sum[b]| / N   over 15 real bins

Per-core device work (125k rows as [125 partitions x 1000 samples x 100 cls]),
balanced across ALL engines (the previous version put everything on DVE):
  - DMA   (sync HWDGE only): 13 tiles, up to 5 MB each
  - ACT   : E = exp(x) in place; later all per-bin statistics via
            activation(Relu/Sign, bias=-C, accum_out=...) which gives a free
            per-partition sum of the activated values
  - DVE   : rowmax(E); rowsum for the small lead tiles; final rowsum over 25
            for the big tiles; recip/eq/mul/stt epilogue per chunk
  - GpSimd: two pairwise-ADD tree levels (100->50->25) in place on each big
            tile, via tensor_tensor(add) -- runs after DVE's rowmax read
            (Pool TT supports add/mult but not max)

Per-bin statistics (accumulated per chunk of samples so they overlap the
main loop instead of forming a serial tail):
  wt'(Cb) = sum relu(conf - Cb)        (ACT, 15 ops: Cb in {0} + C_0..C_13)
  nle_b   = sum (conf <= C_b)          (DVE tensor_scalar accum, 14 ops)
  q'(Tb)  = sum sign(v' - Tb)          (ACT, 15 ops) where
            v' = 2*sign(eg - maxE) - conf  (acc1: -conf; acc0: -2-conf),
            Tb = -C_b for b=0..13 and -1.0 for the total-acc count
Host recovers:
  T = wt'(0);  S_b = T - wt'_b - C_b*(N - nle_b)
  A_b = (N + q'_b)/2   (cumulative acc counts);  diffs give per-bin sums.
C_b is the exact f32 boundary: the largest f32 y with f32(15*y) <= b+1, so
binning matches the reference's ceil(conf*15) up to ~1-sample tie effects
(~1e-6 relative on the final ECE).
"""

import os

import numpy as np

import concourse.bass as bass
import concourse.mybir as mybir
import concourse.tile as tile
from concourse.bass_utils import run_bass_kernel_spmd

F32 = mybir.dt.float32
ALU = mybir.AluOpType
AX = mybir.AxisListType
ACTF = mybir.ActivationFunctionType

N = 1_000_000
C = 100
NCORES = 8
ROWS = N // NCORES          # 125000 rows per core
P = 125                     # SBUF partitions used
SPP = ROWS // P             # 1000 samples per partition

# small tiles at both ends: fast pipeline ramp-up AND a short serial tail
SIZES = [12, 13, 25, 50, 100, 100, 100, 100,   # chunk 0 (500)
         100, 100, 100,                        # chunk 1 (300)
         100, 50, 25, 13, 12]                  # chunk 2 (200)
CHUNKS = [(0, 500), (500, 800), (800, 1000)]
CHUNK_LAST_TILE = [7, 10, 15]
DVE_FULL_K = 25            # tiles with k <= this do the row sum on DVE too
NSLOT = 44                 # 15 wt(ACT relu) + 14 nn(DVE is_le) + 15 av(ACT sign)

LAST_RESULTS = None         # stashed BassKernelResults for test harness


def _bin_thresholds():
    """C_b = largest f32 y such that f32(15*y) <= b+1, for b = 0..14."""
    thr = []
    for b in range(15):
        tgt = np.float32(b + 1)

        def f(v):
            return np.float32(np.float32(15.0) * v)

        y = np.float32((b + 1) / 15.0)
        if f(y) <= tgt:
            while True:
                y2 = np.nextafter(y, np.float32(np.inf))
                if f(y2) <= tgt:
                    y = y2
                else:
                    break
        else:
            while f(y) > tgt:
                y = np.nextafter(y, np.float32(-np.inf))
        thr.append(np.float32(y))
    return thr


THR = _bin_thresholds()                       # 15 values, b = 0..14

# bias constants shipped as a tiny input tensor (the const-AP pool only has
# 0.0/1.0 pre-registered):  [0] = 0.0 (wt base),  [1+b] = -C_b (wt relu),
# [15+b] = +C_b (av sign on v' = 2*sign(d) - conf),  [29] = +1.0 (acc count)
NCONST = 30
CVEC = np.zeros(NCONST, np.float32)
for _b in range(14):
    CVEC[1 + _b] = -THR[_b]
    CVEC[15 + _b] = THR[_b]
CVEC[29] = np.float32(1.0)


def _fix_sync(nc):
    """Instruction encodings only carry 2 sync-command slots (completion
    update takes one), so every instruction should hold <= 1 wait.  Tile's
    sem emission is not transitively minimal, so: (1) drop waits implied
    transitively through other waits / same-engine program order; (2) split
    any leftover multi-wait instruction into a chain of presync drains."""
    import bisect
    import re

    import bass_rust as _br

    TICK = re.compile(r"^(Activation|DVE|PE|Pool|SP|DMAHW\d+|DMASW\d+)_\d+$")
    ASYNC_T = {"InstDMACopy", "InstTriggerDma"}

    insts = []
    for bb in nc.m.functions[0].blocks:
        for ins in bb.instructions:
            insts.append(ins)
    n = len(insts)

    # producer map: tick sem -> sorted cumulative values + producing inst idx
    prod_vals, prod_idx = {}, {}
    own_updates = [[] for _ in range(n)]
    cum = {}
    for idx, ins in enumerate(insts):
        si = ins.sync_info
        if si is None:
            continue
        for u in si.on_update:
            nm = u.ant_name
            if not nm or not TICK.match(nm) or u.update_mode != "sem-inc":
                continue
            v = cum.get(nm, 0) + (u.update_value or 1)
            cum[nm] = v
            prod_vals.setdefault(nm, []).append(v)
            prod_idx.setdefault(nm, []).append(idx)
            own_updates[idx].append((nm, v))

    def producer(nm, val):
        vs = prod_vals.get(nm)
        if not vs:
            return None
        k = bisect.bisect_left(vs, val)
        if k >= len(vs):
            return None
        return prod_idx[nm][k]

    prev_idx = [None] * n
    last = {}
    for idx, ins in enumerate(insts):
        e = str(getattr(ins, "engine", None))
        prev_idx[idx] = last.get(e)
        last[e] = idx

    # before[i]: sem clock guaranteed when inst i issues (incl its waits)
    # after[i]: clock guaranteed when inst i COMPLETES (incl own updates)
    before = [None] * n
    after = [None] * n

    def wait_producers(i):
        si = insts[i].sync_info
        out = []
        for w in (si.on_wait if si else []):
            pi = None
            if w.ant_name and TICK.match(w.ant_name):
                pi = producer(w.ant_name, w.wait_value)
                if pi == i:
                    pi = None
            out.append((w, pi))
        return out

    def compute(idx):
        stack = [idx]
        while stack:
            i = stack[-1]
            if after[i] is not None:
                stack.pop()
                continue
            deps = []
            p = prev_idx[i]
            if p is not None and after[p] is None:
                deps.append(p)
            wps = wait_producers(i)
            for w, pi in wps:
                if pi is not None and after[pi] is None:
                    deps.append(pi)
            if deps:
                stack.extend(deps)
                continue
            stack.pop()
            c = {}
            if p is not None:
                src = before[p] if type(insts[p]).__name__ in ASYNC_T else after[p]
                for s, v in src.items():
                    if c.get(s, -1) < v:
                        c[s] = v
            for w, pi in wps:
                if pi is not None:
                    for s, v in after[pi].items():
                        if c.get(s, -1) < v:
                            c[s] = v
                if w.ant_name and TICK.match(w.ant_name):
                    if c.get(w.ant_name, -1) < w.wait_value:
                        c[w.ant_name] = w.wait_value
            before[i] = c
            a = dict(c)
            for nm, v in own_updates[i]:
                if a.get(nm, -1) < v:
                    a[nm] = v
            after[i] = a

    for i in range(n):
        compute(i)

    # pass 1: transitive reduction of each instruction's wait list
    for i, ins in enumerate(insts):
        si = ins.sync_info
        if si is None or len(si.on_wait) <= 1:
            continue
        if type(ins).__name__ == "InstEventSemaphore":
            continue
        waits = list(si.on_wait)
        p = prev_idx[i]
        base = {}
        if p is not None:
            src = before[p] if type(insts[p]).__name__ in ASYNC_T else after[p]
            base.update(src)
        closures = []
        for w in waits:
            cl = {}
            if w.ant_name and TICK.match(w.ant_name):
                pi = producer(w.ant_name, w.wait_value)
                if pi is not None and pi != i:
                    cl.update(after[pi])
                if cl.get(w.ant_name, -1) < w.wait_value:
                    cl[w.ant_name] = w.wait_value
            closures.append(cl)
        kept = []
        kept_cl = dict(base)
        for j, w in enumerate(waits):
            nm = w.ant_name
            if not (nm and TICK.match(nm)):
                kept.append(w)
                continue
            cov = dict(kept_cl)
            for j2 in range(j + 1, len(waits)):
                for s, v in closures[j2].items():
                    if cov.get(s, -1) < v:
                        cov[s] = v
            if cov.get(nm, -1) >= w.wait_value:
                continue
            kept.append(w)
            for s, v in closures[j].items():
                if kept_cl.get(s, -1) < v:
                    kept_cl[s] = v
        if len(kept) != len(waits):
            si.on_wait = kept
            ins.sync_info = si

    # pass 2: split any instruction still carrying > 1 wait into a chain of
    # same-engine presync drains (each drain fits a single sync command)
    for bb in nc.m.functions[0].blocks:
        while True:
            insns = list(bb.instructions)
            target = None
            for idx, ins in enumerate(insns):
                si = ins.sync_info
                if si is None:
                    continue
                if len(si.on_wait) > 1:
                    target = (idx, ins)
                    break
            if target is None:
                break
            idx, ins = target
            si = ins.sync_info
            waits = list(si.on_wait)
            if type(ins).__name__ == "InstDrain":
                room = max(0, 1 - len(si.on_update))
            else:
                room = 1
            keep, extra = waits[len(waits) - room:], waits[: len(waits) - room]
            pos = idx
            for i, w in enumerate(extra):
                nd = mybir.InstDrain(
                    name=f"{ins.name}-presync{i}", ins=[], outs=[],
                    bass_is_fusable=False,
                )
                nd.engine = ins.engine
                nd.sync_info = _br.SyncInfo(on_wait=[w], on_update=[])
                nc.register_instruction(nd, overwrite=True)
                bb.instructions.insert(pos, nd)
                pos += 1
            si.on_wait = keep
            ins.sync_info = si


def _build():
    nc = bass.Bass(trn_type="TRN2")
    x = nc.dram_tensor("x", [P, SPP * C], F32, kind="ExternalInput")
    g = nc.dram_tensor("g", [P, SPP], F32, kind="ExternalInput")
    cst = nc.dram_tensor("cst", [P, NCONST], F32, kind="ExternalInput")
    st = nc.dram_tensor("st", [P, 3 * NSLOT], F32, kind="ExternalOutput")

    X = x[:, :].rearrange("p (k c) -> p k c", c=C)  # [125, 1000, 100]

    with tile.TileContext(nc) as tc:
        with (
            tc.tile_pool(name="xin", bufs=4) as xin,
            tc.tile_pool(name="persist", bufs=1) as persist,
        ):
            # per-chunk persistent buffers (separate tiles so later-tile
            # writes never alias earlier chunks' binning reads)
            m_ch = [persist.tile([P, hi - lo], F32, tag=f"m{i}", name=f"m{i}")
                    for i, (lo, hi) in enumerate(CHUNKS)]
            s_ch = [persist.tile([P, hi - lo], F32, tag=f"s{i}", name=f"s{i}")
                    for i, (lo, hi) in enumerate(CHUNKS)]
            eg = persist.tile([P, SPP], F32)
            dump = persist.tile([P, 600], F32)
            stats = persist.tile([P, 3 * NSLOT], F32)
            cst_sb = persist.tile([P, NCONST], F32)

            nc.scalar.dma_start(out=eg[:, :], in_=g[:, :])
            nc.scalar.dma_start(out=cst_sb[:, :], in_=cst[:, :])

            # binning ops, built per chunk when it closes and interleaved
            # into later tiles' ACT / DVE streams so they overlap the loop
            act_q = []          # pending ACT binning thunks
            dve_q = []          # pending DVE binning thunks

            def make_binning(cidx):
                lo, hi = CHUNKS[cidx]
                L = hi - lo
                conf = m_ch[cidx][:, :L]
                v = s_ch[cidx][:, :L]
                dmp = dump[:, 0:L]

                def wt_op(j):
                    bias = cst_sb[:, j:j + 1] if j else cst_sb[:, 0:1]
                    return lambda: nc.scalar.activation(
                        dmp, conf, ACTF.Relu, bias=bias,
                        accum_out=stats[:, cidx * NSLOT + j:cidx * NSLOT + j + 1],
                    )

                def av_op(j):
                    bias = cst_sb[:, 15 + j:16 + j] if j < 14 else cst_sb[:, 29:30]
                    return lambda: nc.scalar.activation(
                        dmp, v, ACTF.Sign, bias=bias,
                        accum_out=stats[:, cidx * NSLOT + 29 + j:cidx * NSLOT + 30 + j],
                    )

                def nn_op(j):
                    return lambda: nc.vector.tensor_scalar(
                        dmp, conf, float(THR[j]), None,
                        op0=ALU.is_le, op1=ALU.add,
                        accum_out=stats[:, cidx * NSLOT + 15 + j:cidx * NSLOT + 16 + j],
                    )

                for j in range(15):
                    act_q.append(wt_op(j))
                    act_q.append(av_op(j))
                for j in range(14):
                    dve_q.append(nn_op(j))

            def drain_queues(nact, ndve):
                for _ in range(min(nact, len(act_q))):
                    act_q.pop(0)()
                for _ in range(min(ndve, len(dve_q))):
                    dve_q.pop(0)()

            off = 0
            ci = 0
            pending_sum = None     # deferred 25-wide row sum (prev tile)
            for t, k in enumerate(SIZES):
                lo, hi = CHUNKS[ci]
                sl = slice(off - lo, off - lo + k)
                off += k
                m_c, s_c = m_ch[ci], s_ch[ci]

                xt = xin.tile([P, 100, C], F32, tag="xt")
                nc.sync.dma_start(out=xt[:, :k, :], in_=X[:, off - k:off, :])
                nc.scalar.activation(xt[:, :k, :], xt[:, :k, :], ACTF.Exp)
                if t == 2:
                    nc.scalar.activation(eg[:, :], eg[:, :], ACTF.Exp)
                drain_queues(5, 0)
                nc.vector.reduce_max(out=m_c[:, sl], in_=xt[:, :k, :], axis=AX.X)
                if k <= DVE_FULL_K:
                    nc.vector.reduce_sum(
                        out=s_c[:, sl], in_=xt[:, :k, :], axis=AX.X
                    )
                else:
                    # pairwise ADD tree on GpSimd (Pool TT supports add, not
                    # max), in place after the DVE rowmax read; the final
                    # 25-wide DVE reduce is DEFERRED one tile so DVE is not
                    # head-of-line blocked waiting on the Pool engine
                    nc.gpsimd.tensor_tensor(
                        xt[:, :k, 0:50], xt[:, :k, 0:50], xt[:, :k, 50:100],
                        op=ALU.add,
                    )
                    nc.gpsimd.tensor_tensor(
                        xt[:, :k, 0:25], xt[:, :k, 0:25], xt[:, :k, 25:50],
                        op=ALU.add,
                    )
                    if pending_sum is not None:
                        pending_sum()
                    pending_sum = (
                        lambda xt=xt, k=k, s_c=s_c, sl=sl:
                        nc.vector.reduce_sum(
                            out=s_c[:, sl], in_=xt[:, :k, 0:25], axis=AX.X
                        )
                    )
                drain_queues(0, 2)

                if t == CHUNK_LAST_TILE[ci]:
                    if pending_sum is not None:
                        pending_sum()
                        pending_sum = None
                    # chunk epilogue: r = 1/S (DVE); d = eg - maxE (DVE,
                    # in place into eg; d <= 0, == 0 iff correct);
                    # sd = sign(d) (ACT, in place); conf = maxE * r (DVE);
                    # v' = 2*sd - conf (DVE)  [acc1: -conf; acc0: -2-conf]
                    L = hi - lo
                    egc = eg[:, lo:hi]
                    nc.vector.reciprocal(s_c[:, :L], s_c[:, :L])
                    nc.vector.tensor_tensor(
                        egc, egc, m_c[:, :L], op=ALU.subtract
                    )
                    nc.scalar.activation(egc, egc, ACTF.Sign)
                    nc.vector.tensor_mul(m_c[:, :L], m_c[:, :L], s_c[:, :L])
                    nc.vector.scalar_tensor_tensor(
                        s_c[:, :L], egc, 2.0, m_c[:, :L],
                        op0=ALU.mult, op1=ALU.subtract,
                    )
                    make_binning(ci)
                    ci = min(ci + 1, len(CHUNKS) - 1)

            drain_queues(len(act_q), len(dve_q))
            nc.sync.dma_start(out=st[:, :], in_=stats[:, :])

    _fix_sync(nc)
    return nc


_NC_CACHE = {}


def _get_nc():
    if "nc" not in _NC_CACHE:
        _NC_CACHE["nc"] = _build()
    return _NC_CACHE["nc"]


def kernel(logits, labels):
    global LAST_RESULTS
    logits = np.ascontiguousarray(np.asarray(logits), dtype=np.float32)
    labels_i = np.asarray(labels).astype(np.int64)
    assert logits.shape == (N, C), logits.shape

    # host-side gather of the label logit (1% of input bytes; the heavy
    # softmax/max/binning all happen on device)
    gvals = logits[np.arange(N), labels_i].astype(np.float32)

    in_maps = []
    for c in range(NCORES):
        sl = slice(c * ROWS, (c + 1) * ROWS)
        in_maps.append(
            {
                "x": logits[sl].reshape(P, SPP * C),
                "g": gvals[sl].reshape(P, SPP),
                "cst": np.tile(CVEC, (P, 1)),
            }
        )

    trace = bool(int(os.environ.get("ECE_TRACE", "0")))
    res = run_bass_kernel_spmd(
        _get_nc(), in_maps, core_ids=list(range(NCORES)), trace=trace
    )
    LAST_RESULTS = res

    tot = np.zeros((3, NSLOT), np.float64)
    for out in res.results:
        tot += out["st"].astype(np.float64).reshape(P, 3, NSLOT).sum(axis=0)
    slots = tot.sum(axis=0)                    # [44] summed over chunks

    wtv = slots[0:15]                          # relu sums: C=0, C_0..C_13
    nle = slots[15:29]                         # direct #(conf <= C_b) counts
    q_av = slots[29:44]                        # sign sums on v'; [14]=acc cnt

    thr64 = np.array([np.float64(t) for t in THR])
    T = wtv[0]
    ngt = N - nle
    S = np.empty(16)
    S[0:14] = T - wtv[1:15] - thr64[0:14] * ngt
    S[14] = T
    S[15] = T
    A = np.empty(16)
    A[0:14] = (N + q_av[0:14]) / 2.0
    A[14] = (N + q_av[14]) / 2.0               # total #acc (v >= 0.5)
    A[15] = A[14]
    conf_sum = np.diff(S, prepend=0.0)
    acc_sum = np.diff(A, prepend=0.0)
    ece = np.abs(conf_sum - acc_sum).sum() / N
    return np.array([ece], dtype=np.float32)


# revision 17
# speedup vs baseline: 1.1105x; 1.0802x over previous
"""ECE loss kernel for Trainium2 (Bass/Tile), data-parallel over 8 NeuronCores.

Math (per sample row of logits[N, C]):
  conf = max softmax(x) = max(E) / sum(E),  E = exp(x)
  acc  = (argmax(x) == label)  via  exp(g) == max(E), g = x[i, label_i]
  ece  = sum_b |conf_sum[b] - acc_sum[b]| / N   over 15 real bins

Per-core device work (125k rows as [125 partitions x 1000 samples x 100 cls]),
balanced across ALL engines (the previous version put everything on DVE):
  - DMA   (sync HWDGE only): 13 tiles, up to 5 MB each
  - ACT   : E = exp(x) in place; later all per-bin statistics via
            activation(Relu/Sign, bias=-C, accum_out=...) which gives a free
            per-partition sum of the activated values
  - DVE   : rowmax(E); rowsum for the small lead tiles; final rowsum over 25
            for the big tiles; recip/eq/mul/stt epilogue per chunk
  - GpSimd: two pairwise-ADD tree levels (100->50->25) in place on each big
            tile, via tensor_tensor(add) -- runs after DVE's rowmax read
            (Pool TT supports add/mult but not max)

Per-bin statistics (accumulated per chunk of samples so they overlap the
main loop instead of forming a serial tail):
  wt'(Cb) = sum relu(conf - Cb)        (ACT, 15 ops: Cb in {0} + C_0..C_13)
  nle_b   = sum (conf <= C_b)          (DVE tensor_scalar accum, 14 ops)
  q'(Tb)  = sum sign(v' - Tb)          (ACT, 15 ops) where
            v' = 2*sign(eg - maxE) - conf  (acc1: -conf; acc0: -2-conf),
            Tb = -C_b for b=0..13 and -1.0 for the total-acc count
Host recovers:
  T = wt'(0);  S_b = T - wt'_b - C_b*(N - nle_b)
  A_b = (N + q'_b)/2   (cumulative acc counts);  diffs give per-bin sums.
C_b is the exact f32 boundary: the largest f32 y with f32(15*y) <= b+1, so
binning matches the reference's ceil(conf*15) up to ~1-sample tie effects
(~1e-6 relative on the final ECE).
"""

import os

import numpy as np

import concourse.bass as bass
import concourse.mybir as mybir
import concourse.tile as tile
from concourse.bass_utils import run_bass_kernel_spmd

F32 = mybir.dt.float32
ALU = mybir.AluOpType
AX = mybir.AxisListType
ACTF = mybir.ActivationFunctionType

N = 1_000_000
C = 100
NCORES = 8
ROWS = N // NCORES          # 125000 rows per core
P = 125                     # SBUF partitions used
SPP = ROWS // P             # 1000 samples per partition

# small tiles at both ends: fast pipeline ramp-up AND a short serial tail
SIZES = [12, 13, 25, 50, 100, 100, 100, 100,   # chunk 0 (500)
         100, 100, 100,                        # chunk 1 (300)
         100, 50, 25, 13, 12]                  # chunk 2 (200)
CHUNKS = [(0, 500), (500, 800), (800, 1000)]
CHUNK_LAST_TILE = [7, 10, 15]
DVE_FULL_K = 25            # tiles with k <= this do the row sum on DVE too
NSLOT = 44                 # 15 wt(ACT relu) + 14 nn(DVE is_le) + 15 av(ACT sign)

LAST_RESULTS = None         # stashed BassKernelResults for test harness


def _bin_thresholds():
    """C_b = largest f32 y such that f32(15*y) <= b+1, for b = 0..14."""
    thr = []
    for b in range(15):
        tgt = np.float32(b + 1)

        def f(v):
            return np.float32(np.float32(15.0) * v)

        y = np.float32((b + 1) / 15.0)
        if f(y) <= tgt:
            while True:
                y2 = np.nextafter(y, np.float32(np.inf))
                if f(y2) <= tgt:
                    y = y2
                else:
                    break
        else:
            while f(y) > tgt:
                y = np.nextafter(y, np.float32(-np.inf))
        thr.append(np.float32(y))
    return thr


THR = _bin_thresholds()                       # 15 values, b = 0..14

# bias constants shipped as a tiny input tensor (the const-AP pool only has
# 0.0/1.0 pre-registered):  [0] = 0.0 (wt base),  [1+b] = -C_b (wt relu),
# [15+b] = +C_b (av sign on v' = 2*sign(d) - conf),  [29] = +1.0 (acc count)
NCONST = 30
CVEC = np.zeros(NCONST, np.float32)
for _b in range(14):
    CVEC[1 + _b] = -THR[_b]
    CVEC[15 + _b] = THR[_b]
CVEC[29] = np.float32(1.0)


def _fix_sync(nc):
    """Instruction encodings only carry 2 sync-command slots (completion
    update takes one), so every instruction should hold <= 1 wait.  Tile's
    sem emission is not transitively minimal, so: (1) drop waits implied
    transitively through other waits / same-engine program order; (2) split
    any leftover multi-wait instruction into a chain of presync drains."""
    import bisect
    import re

    import bass_rust as _br

    TICK = re.compile(r"^(Activation|DVE|PE|Pool|SP|DMAHW\d+|DMASW\d+)_\d+$")
    ASYNC_T = {"InstDMACopy", "InstTriggerDma"}

    insts = []
    for bb in nc.m.functions[0].blocks:
        for ins in bb.instructions:
            insts.append(ins)
    n = len(insts)

    # producer map: tick sem -> sorted cumulative values + producing inst idx
    prod_vals, prod_idx = {}, {}
    own_updates = [[] for _ in range(n)]
    cum = {}
    for idx, ins in enumerate(insts):
        si = ins.sync_info
        if si is None:
            continue
        for u in si.on_update:
            nm = u.ant_name
            if not nm or not TICK.match(nm) or u.update_mode != "sem-inc":
                continue
            v = cum.get(nm, 0) + (u.update_value or 1)
            cum[nm] = v
            prod_vals.setdefault(nm, []).append(v)
            prod_idx.setdefault(nm, []).append(idx)
            own_updates[idx].append((nm, v))

    def producer(nm, val):
        vs = prod_vals.get(nm)
        if not vs:
            return None
        k = bisect.bisect_left(vs, val)
        if k >= len(vs):
            return None
        return prod_idx[nm][k]

    prev_idx = [None] * n
    last = {}
    for idx, ins in enumerate(insts):
        e = str(getattr(ins, "engine", None))
        prev_idx[idx] = last.get(e)
        last[e] = idx

    # before[i]: sem clock guaranteed when inst i issues (incl its waits)
    # after[i]: clock guaranteed when inst i COMPLETES (incl own updates)
    before = [None] * n
    after = [None] * n

    def wait_producers(i):
        si = insts[i].sync_info
        out = []
        for w in (si.on_wait if si else []):
            pi = None
            if w.ant_name and TICK.match(w.ant_name):
                pi = producer(w.ant_name, w.wait_value)
                if pi == i:
                    pi = None
            out.append((w, pi))
        return out

    def compute(idx):
        stack = [idx]
        while stack:
            i = stack[-1]
            if after[i] is not None:
                stack.pop()
                continue
            deps = []
            p = prev_idx[i]
            if p is not None and after[p] is None:
                deps.append(p)
            wps = wait_producers(i)
            for w, pi in wps:
                if pi is not None and after[pi] is None:
                    deps.append(pi)
            if deps:
                stack.extend(deps)
                continue
            stack.pop()
            c = {}
            if p is not None:
                src = before[p] if type(insts[p]).__name__ in ASYNC_T else after[p]
                for s, v in src.items():
                    if c.get(s, -1) < v:
                        c[s] = v
            for w, pi in wps:
                if pi is not None:
                    for s, v in after[pi].items():
                        if c.get(s, -1) < v:
                            c[s] = v
                if w.ant_name and TICK.match(w.ant_name):
                    if c.get(w.ant_name, -1) < w.wait_value:
                        c[w.ant_name] = w.wait_value
            before[i] = c
            a = dict(c)
            for nm, v in own_updates[i]:
                if a.get(nm, -1) < v:
                    a[nm] = v
            after[i] = a

    for i in range(n):
        compute(i)

    # pass 1: transitive reduction of each instruction's wait list
    for i, ins in enumerate(insts):
        si = ins.sync_info
        if si is None or len(si.on_wait) <= 1:
            continue
        if type(ins).__name__ == "InstEventSemaphore":
            continue
        waits = list(si.on_wait)
        p = prev_idx[i]
        base = {}
        if p is not None:
            src = before[p] if type(insts[p]).__name__ in ASYNC_T else after[p]
            base.update(src)
        closures = []
        for w in waits:
            cl = {}
            if w.ant_name and TICK.match(w.ant_name):
                pi = producer(w.ant_name, w.wait_value)
                if pi is not None and pi != i:
                    cl.update(after[pi])
                if cl.get(w.ant_name, -1) < w.wait_value:
                    cl[w.ant_name] = w.wait_value
            closures.append(cl)
        kept = []
        kept_cl = dict(base)
        for j, w in enumerate(waits):
            nm = w.ant_name
            if not (nm and TICK.match(nm)):
                kept.append(w)
                continue
            cov = dict(kept_cl)
            for j2 in range(j + 1, len(waits)):
                for s, v in closures[j2].items():
                    if cov.get(s, -1) < v:
                        cov[s] = v
            if cov.get(nm, -1) >= w.wait_value:
                continue
            kept.append(w)
            for s, v in closures[j].items():
                if kept_cl.get(s, -1) < v:
                    kept_cl[s] = v
        if len(kept) != len(waits):
            si.on_wait = kept
            ins.sync_info = si

    # pass 2: split any instruction still carrying > 1 wait into a chain of
    # same-engine presync drains (each drain fits a single sync command)
    for bb in nc.m.functions[0].blocks:
        while True:
            insns = list(bb.instructions)
            target = None
            for idx, ins in enumerate(insns):
                si = ins.sync_info
                if si is None:
                    continue
                if len(si.on_wait) > 1:
                    target = (idx, ins)
                    break
            if target is None:
                break
            idx, ins = target
            si = ins.sync_info
            waits = list(si.on_wait)
            if type(ins).__name__ == "InstDrain":
                room = max(0, 1 - len(si.on_update))
            else:
                room = 1
            keep, extra = waits[len(waits) - room:], waits[: len(waits) - room]
            pos = idx
            for i, w in enumerate(extra):
                nd = mybir.InstDrain(
                    name=f"{ins.name}-presync{i}", ins=[], outs=[],
                    bass_is_fusable=False,
                )
                nd.engine = ins.engine
                nd.sync_info = _br.SyncInfo(on_wait=[w], on_update=[])
                nc.register_instruction(nd, overwrite=True)
                bb.instructions.insert(pos, nd)
                pos += 1
            si.on_wait = keep
            ins.sync_info = si


def _build():
    nc = bass.Bass(trn_type="TRN2")
    x = nc.dram_tensor("x", [P, SPP * C], F32, kind="ExternalInput")
    g = nc.dram_tensor("g", [P, SPP], F32, kind="ExternalInput")
    cst = nc.dram_tensor("cst", [P, NCONST], F32, kind="ExternalInput")
    st = nc.dram_tensor("st", [P, 3 * NSLOT], F32, kind="ExternalOutput")

    X = x[:, :].rearrange("p (k c) -> p k c", c=C)  # [125, 1000, 100]

    with tile.TileContext(nc) as tc:
        with (
            tc.tile_pool(name="xin", bufs=1) as xin,
            tc.tile_pool(name="persist", bufs=1) as persist,
        ):
            # per-chunk persistent buffers (separate tiles so later-tile
            # writes never alias earlier chunks' binning reads)
            m_ch = [persist.tile([P, hi - lo], F32, tag=f"m{i}", name=f"m{i}")
                    for i, (lo, hi) in enumerate(CHUNKS)]
            s_ch = [persist.tile([P, hi - lo], F32, tag=f"s{i}", name=f"s{i}")
                    for i, (lo, hi) in enumerate(CHUNKS)]
            eg = persist.tile([P, SPP], F32)
            # separate scratch outputs per engine: a shared one creates
            # cross-engine WAW chains that serialize the whole pipeline
            dump_a = persist.tile([P, 600], F32)
            dump_d = persist.tile([P, 600], F32)
            stats = persist.tile([P, 3 * NSLOT], F32)
            cst_sb = persist.tile([P, NCONST], F32)

            nc.scalar.dma_start(out=eg[:, :], in_=g[:, :])
            nc.scalar.dma_start(out=cst_sb[:, :], in_=cst[:, :])

            # binning ops, built per chunk when it closes and interleaved
            # into later tiles' ACT / DVE streams so they overlap the loop
            act_q = []          # pending ACT binning thunks
            dve_q = []          # pending DVE binning thunks

            def make_binning(cidx):
                lo, hi = CHUNKS[cidx]
                L = hi - lo
                conf = m_ch[cidx][:, :L]
                v = s_ch[cidx][:, :L]
                dmp = dump_a[:, 0:L]
                dmp_d = dump_d[:, 0:L]

                def wt_op(j):
                    bias = cst_sb[:, j:j + 1] if j else cst_sb[:, 0:1]
                    return lambda: nc.scalar.activation(
                        dmp, conf, ACTF.Relu, bias=bias,
                        accum_out=stats[:, cidx * NSLOT + j:cidx * NSLOT + j + 1],
                    )

                def av_op(j):
                    bias = cst_sb[:, 15 + j:16 + j] if j < 14 else cst_sb[:, 29:30]
                    return lambda: nc.scalar.activation(
                        dmp, v, ACTF.Sign, bias=bias,
                        accum_out=stats[:, cidx * NSLOT + 29 + j:cidx * NSLOT + 30 + j],
                    )

                def nn_op(j):
                    return lambda: nc.vector.tensor_scalar(
                        dmp_d, conf, float(THR[j]), None,
                        op0=ALU.is_le, op1=ALU.add,
                        accum_out=stats[:, cidx * NSLOT + 15 + j:cidx * NSLOT + 16 + j],
                    )

                for j in range(15):
                    act_q.append(wt_op(j))
                    act_q.append(av_op(j))
                for j in range(14):
                    dve_q.append(nn_op(j))

            def drain_queues(nact, ndve):
                for _ in range(min(nact, len(act_q))):
                    act_q.pop(0)()
                for _ in range(min(ndve, len(dve_q))):
                    dve_q.pop(0)()

            off = 0
            ci = 0
            pending_sum = None     # deferred 25-wide row sum (prev tile)
            for t, k in enumerate(SIZES):
                lo, hi = CHUNKS[ci]
                sl = slice(off - lo, off - lo + k)
                off += k
                m_c, s_c = m_ch[ci], s_ch[ci]

                # explicit per-slot tags: Tile's free-pool reuse is LIFO,
                # which collapses the 4 buffers to ~2 and serializes the
                # pipeline; manual round-robin enforces reuse distance 4
                xt = xin.tile([P, 100, C], F32, tag=f"xt{t % 4}", name=f"xt{t}")
                nc.sync.dma_start(out=xt[:, :k, :], in_=X[:, off - k:off, :])
                nc.scalar.activation(xt[:, :k, :], xt[:, :k, :], ACTF.Exp)
                if t == 2:
                    nc.scalar.activation(eg[:, :], eg[:, :], ACTF.Exp)
                drain_queues(5, 0)
                nc.vector.reduce_max(out=m_c[:, sl], in_=xt[:, :k, :], axis=AX.X)
                if k <= DVE_FULL_K:
                    nc.vector.reduce_sum(
                        out=s_c[:, sl], in_=xt[:, :k, :], axis=AX.X
                    )
                else:
                    # pairwise ADD tree on GpSimd (Pool TT supports add, not
                    # max), in place after the DVE rowmax read; the final
                    # 25-wide DVE reduce is DEFERRED one tile so DVE is not
                    # head-of-line blocked waiting on the Pool engine
                    nc.gpsimd.tensor_tensor(
                        xt[:, :k, 0:50], xt[:, :k, 0:50], xt[:, :k, 50:100],
                        op=ALU.add,
                    )
                    nc.gpsimd.tensor_tensor(
                        xt[:, :k, 0:25], xt[:, :k, 0:25], xt[:, :k, 25:50],
                        op=ALU.add,
                    )
                    if pending_sum is not None:
                        pending_sum()
                    pending_sum = (
                        lambda xt=xt, k=k, s_c=s_c, sl=sl:
                        nc.vector.reduce_sum(
                            out=s_c[:, sl], in_=xt[:, :k, 0:25], axis=AX.X
                        )
                    )
                drain_queues(0, 2)

                if t == CHUNK_LAST_TILE[ci]:
                    if pending_sum is not None:
                        pending_sum()
                        pending_sum = None
                    # chunk epilogue: r = 1/S (DVE); d = eg - maxE (DVE,
                    # in place into eg; d <= 0, == 0 iff correct);
                    # sd = sign(d) (ACT, in place); conf = maxE * r (DVE);
                    # v' = 2*sd - conf (DVE)  [acc1: -conf; acc0: -2-conf]
                    L = hi - lo
                    egc = eg[:, lo:hi]
                    nc.vector.reciprocal(s_c[:, :L], s_c[:, :L])
                    nc.vector.tensor_tensor(
                        egc, egc, m_c[:, :L], op=ALU.subtract
                    )
                    nc.scalar.activation(egc, egc, ACTF.Sign)
                    nc.vector.tensor_mul(m_c[:, :L], m_c[:, :L], s_c[:, :L])
                    nc.vector.scalar_tensor_tensor(
                        s_c[:, :L], egc, 2.0, m_c[:, :L],
                        op0=ALU.mult, op1=ALU.subtract,
                    )
                    make_binning(ci)
                    ci = min(ci + 1, len(CHUNKS) - 1)

            drain_queues(len(act_q), len(dve_q))
            nc.sync.dma_start(out=st[:, :], in_=stats[:, :])

    _fix_sync(nc)
    return nc


_NC_CACHE = {}


def _get_nc():
    if "nc" not in _NC_CACHE:
        _NC_CACHE["nc"] = _build()
    return _NC_CACHE["nc"]


def kernel(logits, labels):
    global LAST_RESULTS
    logits = np.ascontiguousarray(np.asarray(logits), dtype=np.float32)
    labels_i = np.asarray(labels).astype(np.int64)
    assert logits.shape == (N, C), logits.shape

    # host-side gather of the label logit (1% of input bytes; the heavy
    # softmax/max/binning all happen on device)
    gvals = logits[np.arange(N), labels_i].astype(np.float32)

    in_maps = []
    for c in range(NCORES):
        sl = slice(c * ROWS, (c + 1) * ROWS)
        in_maps.append(
            {
                "x": logits[sl].reshape(P, SPP * C),
                "g": gvals[sl].reshape(P, SPP),
                "cst": np.tile(CVEC, (P, 1)),
            }
        )

    trace = bool(int(os.environ.get("ECE_TRACE", "0")))
    res = run_bass_kernel_spmd(
        _get_nc(), in_maps, core_ids=list(range(NCORES)), trace=trace
    )
    LAST_RESULTS = res

    tot = np.zeros((3, NSLOT), np.float64)
    for out in res.results:
        tot += out["st"].astype(np.float64).reshape(P, 3, NSLOT).sum(axis=0)
    slots = tot.sum(axis=0)                    # [44] summed over chunks

    wtv = slots[0:15]                          # relu sums: C=0, C_0..C_13
    nle = slots[15:29]                         # direct #(conf <= C_b) counts
    q_av = slots[29:44]                        # sign sums on v'; [14]=acc cnt

    thr64 = np.array([np.float64(t) for t in THR])
    T = wtv[0]
    ngt = N - nle
    S = np.empty(16)
    S[0:14] = T - wtv[1:15] - thr64[0:14] * ngt
    S[14] = T
    S[15] = T
    A = np.empty(16)
    A[0:14] = (N + q_av[0:14]) / 2.0
    A[14] = (N + q_av[14]) / 2.0               # total #acc (v >= 0.5)
    A[15] = A[14]
    conf_sum = np.diff(S, prepend=0.0)
    acc_sum = np.diff(A, prepend=0.0)
    ece = np.abs(conf_sum - acc_sum).sum() / N
    return np.array([ece], dtype=np.float32)
